# revision 31
# baseline (speedup 1.0000x reference)
"""BLSTM (embed -> bidirectional LSTM -> vocab projection) on 8 trn2 NeuronCores.

Strategy (SPMD, one program on all 8 cores; per-core *data* differs):
  - cores 0-3 run the forward LSTM scan, cores 4-7 the backward scan
    (backward = forward program on time-reversed token indices).
  - scan computes gates transposed ([128 gate-dims, 8 batch]) so the
    elementwise chain uses all 128 partitions with tiny free dims; the
    x-projection term is pre-accumulated into PSUM with identity matmuls.
  - hidden states are exchanged in NCHK chunks via AllGather over pairs
    [c, c+4]; vocab-sharded projection of each 512-token block starts as
    soon as both directions' chunks for it have arrived, overlapping the
    remaining scan (projection fills PE gaps).
  - core c computes logits[:, :, c*Vs:(c+1)*Vs].

Numerics: fp16 matmul operands, fp32 PSUM/cell-state/activations.
"""
import sys
import numpy as np

sys.path.insert(0, "/opt/trn_rl_repo")

import concourse.bass as bass
import concourse.mybir as mybir
import concourse.tile as tile
from concourse import bacc
from concourse.bass_utils import run_bass_kernel_spmd
from concourse.masks import make_identity

f16 = mybir.dt.float16
f32 = mybir.dt.float32
i32 = mybir.dt.int32

# full problem dims
V_FULL, E_FULL, H_FULL = 32000, 64, 256
B_FULL, T_FULL = 8, 512
NCORES = 8

_PROGRAM_CACHE = {}


def build_program(V, E, H, B, T):
    """One SPMD Bass program for all 8 cores."""
    BT = B * T                 # tokens
    NT = BT // 128             # 128-token tiles
    KC = H // 128              # h chunks (contraction tiles for Wh)
    GT = 4 * H // 128          # gate tiles of 128 gate-dims
    Vs = V // NCORES           # per-core vocab slice
    KC2 = 2 * H // 128         # contraction tiles for W_out
    NKV = (Vs + 499) // 500    # ~500-wide vocab chunks per core
    VC = Vs // NKV             # vocab chunk width
    NCHK = 8 if T % 8 == 0 and (T // 8 * B) % 128 == 0 else 1
    CH = T // NCHK             # steps per chunk
    CHB = CH * B               # tokens per chunk
    MTB = CHB // 128           # 128-token tiles per block
    assert BT % 128 == 0 and H % 128 == 0 and Vs % NKV == 0

    nc = bacc.Bacc("TRN2", target_bir_lowering=False, debug=False,
                   num_devices=NCORES)

    emb = nc.dram_tensor("emb", [V, E], f16, kind="ExternalInput").ap()
    idx = nc.dram_tensor("idx", [128, NT], i32, kind="ExternalInput").ap()
    # wi carries the gate bias as an extra contraction row (eT gets a ones row)
    wi = nc.dram_tensor("wi", [E + 1, 4 * H], f16, kind="ExternalInput").ap()
    wh = nc.dram_tensor("wh", [128, KC * GT * 128], f16, kind="ExternalInput").ap()
    wout = nc.dram_tensor("wout", [128, KC2 * Vs], f16, kind="ExternalInput").ap()
    logits = nc.dram_tensor("logits", [BT, Vs], f32, kind="ExternalOutput").ap()

    nfi = GT // 2 * B
    ng = GT // 4 * B

    with tile.TileContext(nc) as tc:
        with (
            tc.tile_pool(name="const", bufs=1) as constp,
            tc.tile_pool(name="dram", bufs=1, space="DRAM") as dram,
            tc.tile_pool(name="big", bufs=1) as big,
            tc.tile_pool(name="work", bufs=1) as work,
            tc.tile_pool(name="chain", bufs=3) as chain,
            tc.tile_pool(name="ost", bufs=3) as ost,
            tc.tile_pool(name="p1ps", bufs=2, space="PSUM") as p1ps,
            tc.tile_pool(name="gps", bufs=1, space="PSUM") as gps,
            tc.tile_pool(name="pj", bufs=2, space="PSUM") as pj,
        ):
            idx_sb = constp.tile([128, NT], i32)
            nc.sync.dma_start(idx_sb[:], idx)
            ident = constp.tile([128, 128], f16)
            make_identity(nc, ident[:])
            wi_sb = constp.tile([E + 1, 4 * H], f16)
            nc.sync.dma_start(wi_sb[:], wi)
            wh_sb = constp.tile([128, KC * GT * 128], f16)
            nc.sync.dma_start(wh_sb[:], wh)
            wout_sb = constp.tile([128, KC2 * Vs], f16)

            hs_dram = [dram.tile([128, KC * CHB], f16, name=f"hsd{k}")
                       for k in range(NCHK)]
            h2_dram = [dram.tile([2, 128, KC * CHB], f16, name=f"h2d{k}")
                       for k in range(NCHK)]

            # ---- phase 1/2: LSTM scan with chunked hidden-state exchange ---
            # gates_t = Wi'^T e'_t  +  Wh^T h_{t-1}, accumulated in PSUM.
            # Each chunk's embedding gather+transpose is emitted one chunk
            # ahead of use so it overlaps the previous chunk's scan.
            eT = [None] * NCHK

            def emit_chunk_embed(k):
                eT[k] = work.tile([E + 1, CHB], f16, tag="eT", bufs=3,
                                  name=f"eT{k}")
                nc.vector.memset(eT[k][E:E + 1, :], 1.0)
                for jl in range(CHB // 128):
                    j = k * MTB + jl
                    g_sb = work.tile([128, E], f16, tag="gath", bufs=3,
                                     name=f"gath{j}")
                    nc.gpsimd.indirect_dma_start(
                        out=g_sb[:], out_offset=None, in_=emb,
                        in_offset=bass.IndirectOffsetOnAxis(
                            ap=idx_sb[:, j:j + 1], axis=0),
                    )
                    tp_ps = p1ps.tile([E, 128], f16, tag="p1",
                                      name=f"tp{j}")
                    nc.tensor.transpose(out=tp_ps[:], in_=g_sb[:],
                                        identity=ident[:])
                    nc.vector.tensor_copy(
                        out=eT[k][0:E, jl * 128:(jl + 1) * 128], in_=tp_ps[:])

            emit_chunk_embed(0)
            c_sb = big.tile([128, KC * B], f32)
            hsT = [None] * NCHK
            for i in range(T):
                k = i // CH
                il = i % CH
                if il == 0:
                    hsT[k] = work.tile([128, KC * CHB], f16, tag="hst",
                                       bufs=3, name=f"hst{k}")
                    if k + 1 < NCHK:
                        emit_chunk_embed(k + 1)
                bank_fi = gps.tile([128, nfi], f32, tag="bfi", bufs=2,
                                   name=f"bfi{i}")
                bank_g = gps.tile([128, ng], f32, tag="bg", bufs=1,
                                  name=f"bg{i}")
                bank_o = gps.tile([128, ng], f32, tag="bo", bufs=1,
                                  name=f"bo{i}")
                e_sl = eT[k][:, il * B:(il + 1) * B]
                for gt in range(GT):
                    # start=True clears the whole bank, so only the first
                    # matmul per bank sets it; later slices overwrite via
                    # per-element has_written bits, then Wh accumulates.
                    if gt < GT // 2:
                        dst = bank_fi[:, gt * B:(gt + 1) * B]
                        first = gt == 0
                    elif gt < GT // 2 + GT // 4:
                        g0 = gt - GT // 2
                        dst = bank_g[:, g0 * B:(g0 + 1) * B]
                        first = g0 == 0
                    else:
                        g0 = gt - GT // 2 - GT // 4
                        dst = bank_o[:, g0 * B:(g0 + 1) * B]
                        first = g0 == 0
                    last = gt in (GT // 2 - 1, GT // 2 + GT // 4 - 1, GT - 1)
                    nc.tensor.matmul(dst,
                                     wi_sb[:, gt * 128:(gt + 1) * 128],
                                     e_sl, start=first,
                                     stop=(i == 0 and last),
                                     skip_group_check=True)
                if i > 0:
                    kp, ilp = (i - 1) // CH, (i - 1) % CH
                    for gt in range(GT):
                        if gt < GT // 2:
                            dst = bank_fi[:, gt * B:(gt + 1) * B]
                        elif gt < GT // 2 + GT // 4:
                            g0 = gt - GT // 2
                            dst = bank_g[:, g0 * B:(g0 + 1) * B]
                        else:
                            g0 = gt - GT // 2 - GT // 4
                            dst = bank_o[:, g0 * B:(g0 + 1) * B]
                        for kc in range(KC):
                            nc.tensor.matmul(
                                dst,
                                wh_sb[:, (gt * KC + kc) * 128:
                                      (gt * KC + kc + 1) * 128],
                                hsT[kp][:, kc * CHB + ilp * B:
                                        kc * CHB + (ilp + 1) * B],
                                start=False, stop=(kc == KC - 1),
                                skip_group_check=True)
                fi_sb = chain.tile([128, nfi], f32, tag="fi")
                nc.scalar.activation(fi_sb[:], bank_fi[:],
                                     mybir.ActivationFunctionType.Sigmoid)
                g_sb2 = chain.tile([128, ng], f32, tag="g")
                nc.scalar.activation(g_sb2[:], bank_g[:],
                                     mybir.ActivationFunctionType.Tanh)
                o_sb = chain.tile([128, ng], f32, tag="o")
                nc.scalar.activation(o_sb[:], bank_o[:],
                                     mybir.ActivationFunctionType.Sigmoid)
                if i == 0:
                    nc.vector.tensor_mul(out=c_sb[:], in0=fi_sb[:, ng:2 * ng],
                                         in1=g_sb2[:])
                else:
                    ig = chain.tile([128, ng], f32, tag="ig")
                    nc.vector.tensor_mul(out=ig[:], in0=fi_sb[:, ng:2 * ng],
                                         in1=g_sb2[:])
                    fc = chain.tile([128, ng], f32, tag="fc")
                    nc.vector.tensor_mul(out=fc[:], in0=fi_sb[:, 0:ng],
                                         in1=c_sb[:])
                    nc.vector.tensor_add(out=c_sb[:], in0=ig[:], in1=fc[:])
                tc_sb = chain.tile([128, ng], f32, tag="tc")
                nc.scalar.activation(tc_sb[:], c_sb[:],
                                     mybir.ActivationFunctionType.Tanh)
                nc.vector.tensor_mul(
                    out=hsT[k][:].rearrange("p (q t) -> p q t",
                                            q=KC)[:, :, il * B:(il + 1) * B],
                    in0=o_sb[:].rearrange("p (q b) -> p q b", q=KC),
                    in1=tc_sb[:].rearrange("p (q b) -> p q b", q=KC))
                if il == CH - 1:
                    # chunk complete: stage to DRAM + exchange with partner
                    nc.sync.dma_start(hs_dram[k][:], hsT[k][:])
                    nc.gpsimd.collective_compute(
                        "AllGather", mybir.AluOpType.bypass,
                        replica_groups=[[c, c + 4] for c in range(4)],
                        ins=[hs_dram[k].opt()], outs=[h2_dram[k].opt()],
                    )

            # ---- phase 3: vocab projection per token block -----------------
            # block j needs fwd chunk j + bwd chunk NCHK-1-j; emit blocks in
            # readiness order. (Emitted after the scan so Tile gives the scan
            # higher priority; these fill engine gaps as chunks arrive.)
            nc.sync.dma_start(wout_sb[:], wout)
            order = []
            for k in range(NCHK):
                for j in {min(k, NCHK - 1 - k), max(k, NCHK - 1 - k)}:
                    if max(j, NCHK - 1 - j) == k:
                        order.append(j)
            for j in order:
                jb = NCHK - 1 - j
                h2b = work.tile([128, 2 * KC * CHB], f16, tag="h2b", bufs=4,
                                name=f"h2b{j}")
                nc.sync.dma_start(h2b[:, 0:KC * CHB], h2_dram[j][0])
                stage = work.tile([128, KC * CHB], f16, tag="stg", bufs=2,
                                  name=f"stg{j}")
                nc.sync.dma_start(stage[:], h2_dram[jb][1])
                # bwd chunk was scanned on reversed time: un-reverse within
                # the chunk while copying into the block tile
                nc.gpsimd.tensor_copy(
                    out=h2b[:, KC * CHB:2 * KC * CHB].rearrange(
                        "p (q t b) -> p q t b", q=KC, b=B),
                    in_=stage[:].rearrange(
                        "p (q t b) -> p q t b", q=KC, b=B)[:, :, ::-1, :])
                vc = VC
                nkv = Vs // vc
                for ml in range(MTB):
                    mt = j * MTB + ml
                    for nk in range(nkv):
                        bank = pj.tile([128, vc], f32, tag="pj",
                                       name=f"pj{mt}_{nk}")
                        for kc in range(KC2):
                            nc.tensor.matmul(
                                bank[:],
                                h2b[:, kc * CHB + ml * 128:
                                    kc * CHB + (ml + 1) * 128],
                                wout_sb[:, kc * Vs + nk * vc:
                                        kc * Vs + (nk + 1) * vc],
                                start=(kc == 0), stop=(kc == KC2 - 1))
                        # PSUM -> SBUF -> DRAM; alternate copy engine to
                        # halve per-engine load (b_out is added host-side in
                        # the rare case it is nonzero)
                        out_sb = ost.tile([128, vc], f32, tag="ot",
                                          name=f"ot{mt}_{nk}")
                        if nk % 2 == 0:
                            nc.vector.tensor_copy(out=out_sb[:], in_=bank[:])
                        else:
                            nc.scalar.copy(out=out_sb[:], in_=bank[:])
                        nc.sync.dma_start(
                            logits[mt * 128:(mt + 1) * 128,
                                   nk * vc:(nk + 1) * vc],
                            out_sb[:])

    nc.compile()
    return nc


def _prep_inputs(x, emb, Wi, Wh, b, W_out, b_out, core, V, E, H, B, T):
    """Per-core input arrays for the SPMD program."""
    BT = B * T
    NT = BT // 128
    KC = H // 128
    GT = 4 * H // 128
    Vs = V // NCORES
    KC2 = 2 * H // 128
    fwd = core < 4
    xs = x if fwd else x[:, ::-1]
    idx = np.ascontiguousarray(xs.T.reshape(NT, 128).T.astype(np.int32))
    wh_arr = np.ascontiguousarray(
        Wh.reshape(KC, 128, GT, 128).transpose(1, 2, 0, 3)
        .reshape(128, GT * KC * 128).astype(np.float16))
    wi_aug = np.vstack([Wi, b[None, :]])
    lo = core * Vs
    wout_arr = np.ascontiguousarray(
        W_out[:, lo:lo + Vs].reshape(KC2, 128, Vs).transpose(1, 0, 2)
        .reshape(128, KC2 * Vs).astype(np.float16))
    return {
        "emb": emb.astype(np.float16),
        "idx": idx,
        "wi": wi_aug.astype(np.float16),
        "wh": wh_arr,
        "wout": wout_arr,
    }


def run(x, emb, Wi_f, Wh_f, b_f, Wi_b, Wh_b, b_b, W_out, b_out,
        V, E, H, B, T):
    key = (V, E, H, B, T)
    if key not in _PROGRAM_CACHE:
        _PROGRAM_CACHE[key] = build_program(V, E, H, B, T)
    nc = _PROGRAM_CACHE[key]

    in_maps = []
    for c in range(NCORES):
        if c < 4:
            m = _prep_inputs(x, emb, Wi_f, Wh_f, b_f, W_out, b_out,
                             c, V, E, H, B, T)
        else:
            m = _prep_inputs(x, emb, Wi_b, Wh_b, b_b, W_out, b_out,
                             c, V, E, H, B, T)
        in_maps.append(m)

    res = run_bass_kernel_spmd(nc, in_maps, list(range(NCORES)))

    Vs = V // NCORES
    out = np.empty((B, T, V), dtype=np.float32)
    for c in range(NCORES):
        sl = res.results[c]["logits"].reshape(T, B, Vs).transpose(1, 0, 2)
        out[:, :, c * Vs:(c + 1) * Vs] = sl
    if np.any(b_out):
        out += b_out.astype(np.float32)
    return out


def kernel(x, emb, Wi_f, Wh_f, b_f, Wi_b, Wh_b, b_b, W_out, b_out):
    return run(np.asarray(x), np.asarray(emb), np.asarray(Wi_f),
               np.asarray(Wh_f), np.asarray(b_f), np.asarray(Wi_b),
               np.asarray(Wh_b), np.asarray(b_b), np.asarray(W_out),
               np.asarray(b_out), V_FULL, E_FULL, H_FULL, B_FULL, T_FULL)


# revision 32
# speedup vs baseline: 1.0138x; 1.0138x over previous
"""BLSTM (embed -> bidirectional LSTM -> vocab projection) on 8 trn2 NeuronCores.

Strategy (SPMD, one program on all 8 cores; per-core *data* differs):
  - cores 0-3 run the forward LSTM scan, cores 4-7 the backward scan
    (backward = forward program on time-reversed token indices).
  - scan computes gates transposed ([128 gate-dims, 8 batch]) so the
    elementwise chain uses all 128 partitions with tiny free dims; the
    x-projection term is pre-accumulated into PSUM with identity matmuls.
  - hidden states are exchanged in NCHK chunks via AllGather over pairs
    [c, c+4]; vocab-sharded projection of each 512-token block starts as
    soon as both directions' chunks for it have arrived, overlapping the
    remaining scan (projection fills PE gaps).
  - core c computes logits[:, :, c*Vs:(c+1)*Vs].

Numerics: fp16 matmul operands, fp32 PSUM/cell-state/activations.
"""
import sys
import numpy as np

sys.path.insert(0, "/opt/trn_rl_repo")

import concourse.bass as bass
import concourse.mybir as mybir
import concourse.tile as tile
from concourse import bacc
from concourse.bass_utils import run_bass_kernel_spmd
from concourse.masks import make_identity

f16 = mybir.dt.float16
f32 = mybir.dt.float32
i32 = mybir.dt.int32

# full problem dims
V_FULL, E_FULL, H_FULL = 32000, 64, 256
B_FULL, T_FULL = 8, 512
NCORES = 8

_PROGRAM_CACHE = {}


def build_program(V, E, H, B, T):
    """One SPMD Bass program for all 8 cores."""
    BT = B * T                 # tokens
    NT = BT // 128             # 128-token tiles
    KC = H // 128              # h chunks (contraction tiles for Wh)
    GT = 4 * H // 128          # gate tiles of 128 gate-dims
    Vs = V // NCORES           # per-core vocab slice
    KC2 = 2 * H // 128         # contraction tiles for W_out
    NKV = (Vs + 499) // 500    # ~500-wide vocab chunks per core
    VC = Vs // NKV             # vocab chunk width
    NCHK = 16 if T % 16 == 0 and (T // 16 * B) % 128 == 0 else 1
    CH = T // NCHK             # steps per chunk
    CHB = CH * B               # tokens per chunk
    MTB = CHB // 128           # 128-token tiles per block
    assert BT % 128 == 0 and H % 128 == 0 and Vs % NKV == 0

    nc = bacc.Bacc("TRN2", target_bir_lowering=False, debug=False,
                   num_devices=NCORES)

    emb = nc.dram_tensor("emb", [V, E], f16, kind="ExternalInput").ap()
    idx = nc.dram_tensor("idx", [128, NT], i32, kind="ExternalInput").ap()
    # wi carries the gate bias as an extra contraction row (eT gets a ones row)
    wi = nc.dram_tensor("wi", [E + 1, 4 * H], f16, kind="ExternalInput").ap()
    wh = nc.dram_tensor("wh", [128, KC * GT * 128], f16, kind="ExternalInput").ap()
    wout = nc.dram_tensor("wout", [128, KC2 * Vs], f16, kind="ExternalInput").ap()
    logits = nc.dram_tensor("logits", [BT, Vs], f32, kind="ExternalOutput").ap()

    nfi = GT // 2 * B
    ng = GT // 4 * B

    with tile.TileContext(nc) as tc:
        with (
            tc.tile_pool(name="const", bufs=1) as constp,
            tc.tile_pool(name="dram", bufs=1, space="DRAM") as dram,
            tc.tile_pool(name="big", bufs=1) as big,
            tc.tile_pool(name="work", bufs=1) as work,
            tc.tile_pool(name="chain", bufs=3) as chain,
            tc.tile_pool(name="ost", bufs=3) as ost,
            tc.tile_pool(name="p1ps", bufs=2, space="PSUM") as p1ps,
            tc.tile_pool(name="gps", bufs=1, space="PSUM") as gps,
            tc.tile_pool(name="pj", bufs=2, space="PSUM") as pj,
        ):
            idx_sb = constp.tile([128, NT], i32)
            nc.sync.dma_start(idx_sb[:], idx)
            ident = constp.tile([128, 128], f16)
            make_identity(nc, ident[:])
            wi_sb = constp.tile([E + 1, 4 * H], f16)
            nc.sync.dma_start(wi_sb[:], wi)
            wh_sb = constp.tile([128, KC * GT * 128], f16)
            nc.sync.dma_start(wh_sb[:], wh)
            wout_sb = constp.tile([128, KC2 * Vs], f16)

            hs_dram = [dram.tile([128, KC * CHB], f16, name=f"hsd{k}")
                       for k in range(NCHK)]
            h2_dram = [dram.tile([2, 128, KC * CHB], f16, name=f"h2d{k}")
                       for k in range(NCHK)]

            # ---- phase 1/2: LSTM scan with chunked hidden-state exchange ---
            # gates_t = Wi'^T e'_t  +  Wh^T h_{t-1}, accumulated in PSUM.
            # Each chunk's embedding gather+transpose is emitted one chunk
            # ahead of use so it overlaps the previous chunk's scan.
            eT = [None] * NCHK

            def emit_chunk_embed(k):
                eT[k] = work.tile([E + 1, CHB], f16, tag="eT", bufs=3,
                                  name=f"eT{k}")
                nc.vector.memset(eT[k][E:E + 1, :], 1.0)
                for jl in range(CHB // 128):
                    j = k * MTB + jl
                    g_sb = work.tile([128, E], f16, tag="gath", bufs=3,
                                     name=f"gath{j}")
                    nc.gpsimd.indirect_dma_start(
                        out=g_sb[:], out_offset=None, in_=emb,
                        in_offset=bass.IndirectOffsetOnAxis(
                            ap=idx_sb[:, j:j + 1], axis=0),
                    )
                    tp_ps = p1ps.tile([E, 128], f16, tag="p1",
                                      name=f"tp{j}")
                    nc.tensor.transpose(out=tp_ps[:], in_=g_sb[:],
                                        identity=ident[:])
                    nc.vector.tensor_copy(
                        out=eT[k][0:E, jl * 128:(jl + 1) * 128], in_=tp_ps[:])

            emit_chunk_embed(0)
            c_sb = big.tile([128, KC * B], f32)
            hsT = [None] * NCHK
            for i in range(T):
                k = i // CH
                il = i % CH
                if il == 0:
                    hsT[k] = work.tile([128, KC * CHB], f16, tag="hst",
                                       bufs=3, name=f"hst{k}")
                    if k + 1 < NCHK:
                        emit_chunk_embed(k + 1)
                bank_fi = gps.tile([128, nfi], f32, tag="bfi", bufs=2,
                                   name=f"bfi{i}")
                bank_g = gps.tile([128, ng], f32, tag="bg", bufs=1,
                                  name=f"bg{i}")
                bank_o = gps.tile([128, ng], f32, tag="bo", bufs=1,
                                  name=f"bo{i}")
                e_sl = eT[k][:, il * B:(il + 1) * B]
                for gt in range(GT):
                    # start=True clears the whole bank, so only the first
                    # matmul per bank sets it; later slices overwrite via
                    # per-element has_written bits, then Wh accumulates.
                    if gt < GT // 2:
                        dst = bank_fi[:, gt * B:(gt + 1) * B]
                        first = gt == 0
                    elif gt < GT // 2 + GT // 4:
                        g0 = gt - GT // 2
                        dst = bank_g[:, g0 * B:(g0 + 1) * B]
                        first = g0 == 0
                    else:
                        g0 = gt - GT // 2 - GT // 4
                        dst = bank_o[:, g0 * B:(g0 + 1) * B]
                        first = g0 == 0
                    last = gt in (GT // 2 - 1, GT // 2 + GT // 4 - 1, GT - 1)
                    nc.tensor.matmul(dst,
                                     wi_sb[:, gt * 128:(gt + 1) * 128],
                                     e_sl, start=first,
                                     stop=(i == 0 and last),
                                     skip_group_check=True)
                if i > 0:
                    kp, ilp = (i - 1) // CH, (i - 1) % CH
                    for gt in range(GT):
                        if gt < GT // 2:
                            dst = bank_fi[:, gt * B:(gt + 1) * B]
                        elif gt < GT // 2 + GT // 4:
                            g0 = gt - GT // 2
                            dst = bank_g[:, g0 * B:(g0 + 1) * B]
                        else:
                            g0 = gt - GT // 2 - GT // 4
                            dst = bank_o[:, g0 * B:(g0 + 1) * B]
                        for kc in range(KC):
                            nc.tensor.matmul(
                                dst,
                                wh_sb[:, (gt * KC + kc) * 128:
                                      (gt * KC + kc + 1) * 128],
                                hsT[kp][:, kc * CHB + ilp * B:
                                        kc * CHB + (ilp + 1) * B],
                                start=False, stop=(kc == KC - 1),
                                skip_group_check=True)
                fi_sb = chain.tile([128, nfi], f32, tag="fi")
                nc.scalar.activation(fi_sb[:], bank_fi[:],
                                     mybir.ActivationFunctionType.Sigmoid)
                g_sb2 = chain.tile([128, ng], f32, tag="g")
                nc.scalar.activation(g_sb2[:], bank_g[:],
                                     mybir.ActivationFunctionType.Tanh)
                o_sb = chain.tile([128, ng], f32, tag="o")
                nc.scalar.activation(o_sb[:], bank_o[:],
                                     mybir.ActivationFunctionType.Sigmoid)
                if i == 0:
                    nc.vector.tensor_mul(out=c_sb[:], in0=fi_sb[:, ng:2 * ng],
                                         in1=g_sb2[:])
                else:
                    ig = chain.tile([128, ng], f32, tag="ig")
                    nc.vector.tensor_mul(out=ig[:], in0=fi_sb[:, ng:2 * ng],
                                         in1=g_sb2[:])
                    fc = chain.tile([128, ng], f32, tag="fc")
                    nc.vector.tensor_mul(out=fc[:], in0=fi_sb[:, 0:ng],
                                         in1=c_sb[:])
                    nc.vector.tensor_add(out=c_sb[:], in0=ig[:], in1=fc[:])
                tc_sb = chain.tile([128, ng], f32, tag="tc")
                nc.scalar.activation(tc_sb[:], c_sb[:],
                                     mybir.ActivationFunctionType.Tanh)
                nc.vector.tensor_mul(
                    out=hsT[k][:].rearrange("p (q t) -> p q t",
                                            q=KC)[:, :, il * B:(il + 1) * B],
                    in0=o_sb[:].rearrange("p (q b) -> p q b", q=KC),
                    in1=tc_sb[:].rearrange("p (q b) -> p q b", q=KC))
                if il == CH - 1:
                    # chunk complete: stage to DRAM + exchange with partner
                    nc.sync.dma_start(hs_dram[k][:], hsT[k][:])
                    nc.gpsimd.collective_compute(
                        "AllGather", mybir.AluOpType.bypass,
                        replica_groups=[[c, c + 4] for c in range(4)],
                        ins=[hs_dram[k].opt()], outs=[h2_dram[k].opt()],
                    )

            # ---- phase 3: vocab projection per token block -----------------
            # block j needs fwd chunk j + bwd chunk NCHK-1-j; emit blocks in
            # readiness order. (Emitted after the scan so Tile gives the scan
            # higher priority; these fill engine gaps as chunks arrive.)
            nc.sync.dma_start(wout_sb[:], wout)
            order = []
            for k in range(NCHK):
                for j in {min(k, NCHK - 1 - k), max(k, NCHK - 1 - k)}:
                    if max(j, NCHK - 1 - j) == k:
                        order.append(j)
            for j in order:
                jb = NCHK - 1 - j
                h2b = work.tile([128, 2 * KC * CHB], f16, tag="h2b", bufs=4,
                                name=f"h2b{j}")
                nc.sync.dma_start(h2b[:, 0:KC * CHB], h2_dram[j][0])
                stage = work.tile([128, KC * CHB], f16, tag="stg", bufs=2,
                                  name=f"stg{j}")
                nc.sync.dma_start(stage[:], h2_dram[jb][1])
                # bwd chunk was scanned on reversed time: un-reverse within
                # the chunk while copying into the block tile
                nc.gpsimd.tensor_copy(
                    out=h2b[:, KC * CHB:2 * KC * CHB].rearrange(
                        "p (q t b) -> p q t b", q=KC, b=B),
                    in_=stage[:].rearrange(
                        "p (q t b) -> p q t b", q=KC, b=B)[:, :, ::-1, :])
                vc = VC
                nkv = Vs // vc
                for ml in range(MTB):
                    mt = j * MTB + ml
                    for nk in range(nkv):
                        bank = pj.tile([128, vc], f32, tag="pj",
                                       name=f"pj{mt}_{nk}")
                        for kc in range(KC2):
                            nc.tensor.matmul(
                                bank[:],
                                h2b[:, kc * CHB + ml * 128:
                                    kc * CHB + (ml + 1) * 128],
                                wout_sb[:, kc * Vs + nk * vc:
                                        kc * Vs + (nk + 1) * vc],
                                start=(kc == 0), stop=(kc == KC2 - 1))
                        # PSUM -> SBUF -> DRAM; alternate copy engine to
                        # halve per-engine load (b_out is added host-side in
                        # the rare case it is nonzero)
                        out_sb = ost.tile([128, vc], f32, tag="ot",
                                          name=f"ot{mt}_{nk}")
                        if nk % 2 == 0:
                            nc.vector.tensor_copy(out=out_sb[:], in_=bank[:])
                        else:
                            nc.scalar.copy(out=out_sb[:], in_=bank[:])
                        nc.sync.dma_start(
                            logits[mt * 128:(mt + 1) * 128,
                                   nk * vc:(nk + 1) * vc],
                            out_sb[:])

    nc.compile()
    return nc


def _prep_inputs(x, emb, Wi, Wh, b, W_out, b_out, core, V, E, H, B, T):
    """Per-core input arrays for the SPMD program."""
    BT = B * T
    NT = BT // 128
    KC = H // 128
    GT = 4 * H // 128
    Vs = V // NCORES
    KC2 = 2 * H // 128
    fwd = core < 4
    xs = x if fwd else x[:, ::-1]
    idx = np.ascontiguousarray(xs.T.reshape(NT, 128).T.astype(np.int32))
    wh_arr = np.ascontiguousarray(
        Wh.reshape(KC, 128, GT, 128).transpose(1, 2, 0, 3)
        .reshape(128, GT * KC * 128).astype(np.float16))
    wi_aug = np.vstack([Wi, b[None, :]])
    lo = core * Vs
    wout_arr = np.ascontiguousarray(
        W_out[:, lo:lo + Vs].reshape(KC2, 128, Vs).transpose(1, 0, 2)
        .reshape(128, KC2 * Vs).astype(np.float16))
    return {
        "emb": emb.astype(np.float16),
        "idx": idx,
        "wi": wi_aug.astype(np.float16),
        "wh": wh_arr,
        "wout": wout_arr,
    }


def run(x, emb, Wi_f, Wh_f, b_f, Wi_b, Wh_b, b_b, W_out, b_out,
        V, E, H, B, T):
    key = (V, E, H, B, T)
    if key not in _PROGRAM_CACHE:
        _PROGRAM_CACHE[key] = build_program(V, E, H, B, T)
    nc = _PROGRAM_CACHE[key]

    in_maps = []
    for c in range(NCORES):
        if c < 4:
            m = _prep_inputs(x, emb, Wi_f, Wh_f, b_f, W_out, b_out,
                             c, V, E, H, B, T)
        else:
            m = _prep_inputs(x, emb, Wi_b, Wh_b, b_b, W_out, b_out,
                             c, V, E, H, B, T)
        in_maps.append(m)

    res = run_bass_kernel_spmd(nc, in_maps, list(range(NCORES)))

    Vs = V // NCORES
    out = np.empty((B, T, V), dtype=np.float32)
    for c in range(NCORES):
        sl = res.results[c]["logits"].reshape(T, B, Vs).transpose(1, 0, 2)
        out[:, :, c * Vs:(c + 1) * Vs] = sl
    if np.any(b_out):
        out += b_out.astype(np.float32)
    return out


def kernel(x, emb, Wi_f, Wh_f, b_f, Wi_b, Wh_b, b_b, W_out, b_out):
    return run(np.asarray(x), np.asarray(emb), np.asarray(Wi_f),
               np.asarray(Wh_f), np.asarray(b_f), np.asarray(Wi_b),
               np.asarray(Wh_b), np.asarray(b_b), np.asarray(W_out),
               np.asarray(b_out), V_FULL, E_FULL, H_FULL, B_FULL, T_FULL)
